# revision 4
# baseline (speedup 1.0000x reference)
"""Multi-head attention (B=4, S=2048, D=1024, H=16) on 8 Trainium2 cores, v3.

Core c: batch c//2, head-group c%2 (8 heads, 512 proj dims).  v3 fuses all
phases into one dense PE/ACT stream built from single-head attention
iterations (pr, qc, hi):

  - scores: lhsT = packed K^T (both heads' dims, K=128 standard matmul),
    rhs = per-head zero-padded Q^T scratch (QTp; the two scratch buffers
    alternate by head parity so their zero halves persist).  No PE tiling
    modes anywhere (they hold the PE clock gate at 1.2 GHz).
  - AV + softmax denominator via the interleaved ones-column (M=65).
  - PSUM: psS 2x[128,1024] (scores/exp, double-buffered) + psAcc [65,1024]
    (AV accumulator) + psA 2x[128,512] dedicated to filler matmuls = 8 banks.
  - projections (K/Q for next pair, V JIT in pair 0, first half of the
    output projection in pair 3) run as atomic filler chunks in dedicated
    PSUM, so the exp stream's double-buffering is never disturbed.
  - copy-first normalization: l row + unnormalized A^T leave PSUM right
    after the last AV; selector-broadcast + reciprocal + scale run deferred
    inside the next iteration.  Output is written bf16 (host upcasts).
"""

import os
import numpy as np

B, S, D = 4, 2048, 1024
H, DK = 16, 64
P = 128
NCORES = 8
HPC = H // 2
PROJ = HPC * DK
NDM = D // P
NPC = PROJ // P
NSC = S // 512
NSO = S // P
NKC = S // P
VW = DK + 1             # 65 cols per head in V_sb: 64 data + ones

MASK_NEG = -30000.0

_cache = {}


def _build():
    import concourse.bacc as bacc
    import concourse.mybir as mybir
    import concourse.tile as tile
    from contextlib import ExitStack

    f32 = mybir.dt.float32
    bf16 = mybir.dt.bfloat16
    AF = mybir.ActivationFunctionType
    MUL = mybir.AluOpType.mult

    nc = bacc.Bacc("TRN2", target_bir_lowering=False, debug=False,
                   num_devices=NCORES)

    qT = nc.dram_tensor("qT", [D, S], bf16, kind="ExternalInput").ap()
    kT = nc.dram_tensor("kT", [D, S], bf16, kind="ExternalInput").ap()
    vT = nc.dram_tensor("vT", [D, S], bf16, kind="ExternalInput").ap()
    wqr = nc.dram_tensor("wqr", [NPC, P, NDM, P], bf16, kind="ExternalInput").ap()
    wkr = nc.dram_tensor("wkr", [NPC, P, NDM, P], bf16, kind="ExternalInput").ap()
    wv = nc.dram_tensor("wv", [D, PROJ], bf16, kind="ExternalInput").ap()
    wo = nc.dram_tensor("wo", [PROJ, D], bf16, kind="ExternalInput").ap()
    bq2 = nc.dram_tensor("bq2", [P, NPC], f32, kind="ExternalInput").ap()
    bk2 = nc.dram_tensor("bk2", [P, NPC], f32, kind="ExternalInput").ap()
    mb = nc.dram_tensor("mb", [P, NKC], f32, kind="ExternalInput").ap()
    sel = nc.dram_tensor("sel", [P, 128], bf16, kind="ExternalInput").ap()
    vones = nc.dram_tensor("vones", [P, NSO, HPC], bf16,
                           kind="ExternalInput").ap()
    out = nc.dram_tensor("out", [S, D], bf16, kind="ExternalOutput").ap()

    with tile.TileContext(nc) as tc, ExitStack() as ctx:
        cpool = ctx.enter_context(tc.tile_pool(name="const", bufs=1))
        sel_sb = cpool.tile([P, 128], bf16)
        nc.sync.dma_start(sel_sb[:], sel)
        mb_sb = cpool.tile([P, NKC], f32)
        nc.sync.dma_start(mb_sb[:], mb)
        bq_sb = cpool.tile([P, NPC], f32)
        nc.sync.dma_start(bq_sb[:], bq2)
        bk_sb = cpool.tile([P, NPC], f32)
        nc.sync.dma_start(bk_sb[:], bk2)
        Lsb = cpool.tile([P, 1024], bf16)
        nc.gpsimd.memset(Lsb[:], 0.0)
        # PE warm-up: ~30 matmuls on zeros keep the HAM activity window busy
        # during the input DMA wait, so the projection prologue runs at 2.4
        # GHz instead of the cold 1.2 GHz default.  Scoped pool: the bank is
        # returned before the attention PSUM pools are opened.
        with tc.tile_pool(name="warm", bufs=1, space="PSUM") as warm:
            wps = warm.tile([P, 512], f32)
            for _ in range(30):
                nc.tensor.matmul(wps, lhsT=Lsb[:, 0:128], rhs=Lsb[:, 0:512],
                                 start=True, stop=True)
        QTp = [cpool.tile([P, 1024], bf16, name=f"qtp{i}") for i in range(2)]
        nc.gpsimd.memset(QTp[0][:], 0.0)
        nc.gpsimd.memset(QTp[1][:], 0.0)

        rpool = ctx.enter_context(tc.tile_pool(name="res", bufs=1))
        kT_sb = rpool.tile([P, NDM, S], bf16)
        qT_sb = rpool.tile([P, NDM, S], bf16)
        QT_sb = rpool.tile([P, NPC, S], bf16)
        KT_sb = rpool.tile([P, NPC, S], bf16)
        AT_sb = rpool.tile([P, NPC, S], bf16)
        V_sb = rpool.tile([P, NSO, HPC * VW], bf16)
        nc.sync.dma_start(
            V_sb.rearrange("p n (h w) -> p n h w", w=VW)[:, :, :, DK], vones)
        # Input DMAs are emitted later, ordered so the prologue's consumers
        # come first: wk0/wq0 -> kT -> qT(first half) -> wv -> vT(4 chunks)
        # -> qT(second half).

        wpool = ctx.enter_context(tc.tile_pool(name="w", bufs=2))
        stpool = ctx.enter_context(tc.tile_pool(name="st", bufs=2))
        epool = ctx.enter_context(tc.tile_pool(name="expS", bufs=2))
        rcpool = ctx.enter_context(tc.tile_pool(name="rc", bufs=1))
        bspool = ctx.enter_context(tc.tile_pool(name="bs", bufs=1))
        opool = ctx.enter_context(tc.tile_pool(name="ostage", bufs=2))
        psS = ctx.enter_context(tc.tile_pool(name="psS", bufs=2, space="PSUM"))
        psA = ctx.enter_context(tc.tile_pool(name="psA", bufs=2, space="PSUM"))
        psAcc = ctx.enter_context(tc.tile_pool(name="psAcc", bufs=1,
                                               space="PSUM"))

        def kq_fills(pr):
            wk_sb = wpool.tile([P, NDM, P], bf16, tag="w", name="wk_sb")
            nc.sync.dma_start(wk_sb[:], wkr[pr])
            wq_sb = wpool.tile([P, NDM, P], bf16, tag="w", name="wq_sb")
            nc.sync.dma_start(wq_sb[:], wqr[pr])

            def chunk(is_k, sc):
                w_sb = wk_sb if is_k else wq_sb
                src = kT_sb if is_k else qT_sb
                ps = psA.tile([P, 512], f32, tag="a", name="psp")
                for dc in range(NDM):
                    nc.tensor.matmul(
                        ps,
                        lhsT=w_sb[:, dc, :],
                        rhs=src[:, dc, sc * 512:(sc + 1) * 512],
                        start=(dc == 0), stop=(dc == NDM - 1),
                    )
                dst = KT_sb if is_k else QT_sb
                bias = bk_sb if is_k else bq_sb
                nc.vector.tensor_scalar_add(
                    dst[:, pr, sc * 512:(sc + 1) * 512], ps,
                    bias[:, pr:pr + 1])

            return [(lambda is_k=is_k, sc=sc: chunk(is_k, sc))
                    for is_k in (True, False) for sc in range(NSC)]

        def v_chunk(vT_sb, wv_sb, so, half):
            """V for 4 heads (half: 0 = pairs 0-1, 1 = pairs 2-3), one so."""
            ps = psA.tile([P, 512], f32, tag="a", name="psv")
            for dc in range(NDM):
                nc.tensor.matmul(
                    ps[:, 0:256],
                    lhsT=vT_sb[:, dc, so * P:(so + 1) * P],
                    rhs=wv_sb[:, dc, half * 256:(half + 1) * 256],
                    start=(dc == 0), stop=(dc == NDM - 1),
                )
            nc.vector.tensor_copy(
                V_sb[:, so, half * 4 * VW:(half + 1) * 4 * VW].rearrange(
                    "p (h w) -> p h w", w=VW)[:, :, 0:DK],
                ps[:, 0:256].rearrange("p (h w) -> p h w", w=DK))

        def c_fill(wo_sb, so, oc):
            ps = psA.tile([P, 512], f32, tag="a", name="psc")
            for pc in range(NPC):
                nc.tensor.matmul(
                    ps,
                    lhsT=AT_sb[:, pc, so * P:(so + 1) * P],
                    rhs=wo_sb[:, pc, oc * 512:(oc + 1) * 512],
                    start=(pc == 0), stop=(pc == NPC - 1),
                )
            ost = opool.tile([P, 512], bf16, tag="o", name="ost")
            nc.vector.tensor_copy(ost, ps)
            nc.sync.dma_start(
                out[so * P:(so + 1) * P, oc * 512:(oc + 1) * 512], ost)

        pending = []
        norm_state = {}

        def norm_step(step):
            """Deferred normalization, staggered so no single DVE blockage
            exceeds ~3.3us (a monolithic chain delays filler evacuations,
            which hold psA buffers and stall the in-order PE queue).

            step 0: selector-broadcast matmul + stage bc out of PSUM
            step 1/2: reciprocal half + per-head scale of that half
            """
            if not pending:
                return
            pr, qc = pending[0]
            ns = norm_state
            if step == 0:
                bc = psS.tile([P, 1024], f32, tag="s", name="bc")
                for sub in range(2):
                    nc.tensor.matmul(
                        bc[:, sub * 512:(sub + 1) * 512],
                        lhsT=sel_sb[:],
                        rhs=Lsb[:, sub * 512:(sub + 1) * 512],
                        start=True, stop=True,
                    )
                bs = bspool.tile([P, 1024], f32, tag="bs", name="bs")
                nc.vector.tensor_copy(bs, bc)
                ns["bs"] = bs
                ns["rc"] = rcpool.tile([P, 1024], bf16, tag="rc", name="rc")
            else:
                sub = step - 1
                cs = slice(sub * 512, (sub + 1) * 512)
                with nc.allow_low_precision(
                        reason="1/l in bf16 is within rtol"):
                    nc.vector.reciprocal(ns["rc"][:, cs], ns["bs"][:, cs])
                for hi in range(2):
                    lo = hi * 64
                    dst = AT_sb[lo:lo + 64, pr,
                                qc * 1024 + sub * 512:
                                qc * 1024 + (sub + 1) * 512]
                    nc.vector.tensor_tensor(dst, dst,
                                            ns["rc"][lo:lo + 64, cs], MUL)
                if step == 2:
                    pending.pop(0)

        def qtp_prep(pr, qc, hi):
            """Stage the zero-padded per-head Q^T slice for (pr, qc, hi)."""
            qoff = qc * 1024
            nc.vector.tensor_copy(QTp[hi][hi * 64:hi * 64 + 64, :],
                                  QT_sb[hi * 64:hi * 64 + 64, pr,
                                        qoff:qoff + 1024])

        def attn_iter(pr, qc, hi, pre=None, fills=(), fill_at=4, prep=None):
            fills = list(fills)
            h = 2 * pr + hi
            qoff = qc * 1024
            qtp = QTp[hi]
            avs = psAcc.tile([P, 1024], f32, tag="av", name="avs")
            for kc in range(NKC):
                sp = psS.tile([P, 1024], f32, tag="s", name="sp")
                for sub in range(2):
                    nc.tensor.matmul(
                        sp[:, sub * 512:(sub + 1) * 512],
                        lhsT=KT_sb[:, pr, kc * P:(kc + 1) * P],
                        rhs=qtp[:, sub * 512:(sub + 1) * 512],
                        start=True, stop=True,
                    )
                e = epool.tile([P, 1024], bf16, tag="e", name="e")
                nc.scalar.activation(
                    e, sp, AF.Exp,
                    bias=mb_sb[:, kc:kc + 1],
                    scale=float(1.0 / np.sqrt(DK)),
                )
                for sub in range(2):
                    nc.tensor.matmul(
                        avs[0:DK + 1, sub * 512:(sub + 1) * 512],
                        lhsT=V_sb[:, kc, h * VW:(h + 1) * VW],
                        rhs=e[:, sub * 512:(sub + 1) * 512],
                        start=(kc == 0), stop=(kc == NKC - 1),
                    )
                if pre is not None:
                    pre(kc)
                if kc == 3:
                    norm_step(0)
                elif kc == 6:
                    norm_step(1)
                elif kc == 9:
                    norm_step(2)
                if kc == 8 and prep is not None:
                    prep()
                if kc >= fill_at and fills:
                    fills.pop(0)()
            while fills:
                fills.pop(0)()
            # evacuate PSUM in two staged half-copies (the next iteration's
            # AV sub-matmuls each wait only on their own half); fan out off
            # the critical path
            st = stpool.tile([P, 1024], bf16, tag="st", name="st")
            nc.vector.tensor_copy(st[0:DK + 1, 0:512], avs[0:DK + 1, 0:512])
            nc.vector.tensor_copy(st[0:DK + 1, 512:1024],
                                  avs[0:DK + 1, 512:1024])
            nc.vector.tensor_copy(Lsb[32 * hi:32 * hi + 1, :],
                                  st[DK:DK + 1, :])
            nc.vector.tensor_copy(
                AT_sb[64 * hi:64 * hi + 64, pr, qoff:qoff + 1024],
                st[0:DK, :])
            if hi == 1:
                pending.append((pr, qc))

        # ---------------- schedule ----------------
        # DMA order: pair-0 weights, kT, first qT half (prologue deps), then
        # wv + vT in seq chunks (V JIT), then the rest.
        # kT first in the DMA queue (the first big dynamic DMA also pays a
        # ~10us engine-startup penalty; K-proj needs kT + the small weight
        # chunks, so those follow immediately after)
        nc.sync.dma_start(kT_sb[:], kT.rearrange("(o p) s -> p o s", p=P))
        fills0 = kq_fills(0)
        qTr = qT.rearrange("(o p) s -> p o s", p=P)
        nc.sync.dma_start(qT_sb[:, :, 0:1024], qTr[:, :, 0:1024])

        with tc.tile_pool(name="vres", bufs=1) as vpool:
            vT_sb = vpool.tile([P, NDM, S], bf16)
            wv_sb = vpool.tile([P, NDM, PROJ], bf16)
            nc.sync.dma_start(wv_sb[:], wv.rearrange("(o p) n -> p o n", p=P))
            vTr = vT.rearrange("(o p) s -> p o s", p=P)
            for vc in range(4):
                nc.sync.dma_start(vT_sb[:, :, vc * 512:(vc + 1) * 512],
                                  vTr[:, :, vc * 512:(vc + 1) * 512])

            for f in fills0[:6]:      # K sc0-3, Q sc0-1
                f()
            qtp_prep(0, 0, 0)
            v_chunk(vT_sb, wv_sb, 0, 0)
            nc.sync.dma_start(qT_sb[:, :, 1024:2048], qTr[:, :, 1024:2048])

            def pre_v(kc):
                if kc + 1 < NSO:
                    v_chunk(vT_sb, wv_sb, kc + 1, 0)

            fills1 = kq_fills(1)
            fills2 = kq_fills(2)
            vh1 = [(lambda so=so: v_chunk(vT_sb, wv_sb, so, 1))
                   for so in range(NSO)]
            seq1 = [(0, 0, 0), (0, 0, 1), (0, 1, 0), (0, 1, 1),
                    (1, 0, 0), (1, 0, 1), (1, 1, 0), (1, 1, 1)]
            fillmap1 = {
                (0, 0, 0): (fills0[6:], 10),
                (0, 0, 1): ([fills1[0], fills1[1], vh1[0], vh1[1],
                             fills1[2], fills1[3], vh1[2], vh1[3]], 4),
                (0, 1, 0): ([fills1[4], fills1[5], vh1[4], vh1[5],
                             fills1[6], fills1[7], vh1[6], vh1[7]], 4),
                (0, 1, 1): (vh1[8:12], 5),
                (1, 0, 0): ([fills2[0], fills2[1], vh1[12], vh1[13]], 5),
                (1, 0, 1): ([fills2[2], fills2[3], vh1[14], vh1[15]], 5),
                (1, 1, 0): (fills2[4:6], 6),
                (1, 1, 1): (fills2[6:], 6),
            }
            for i, (pr, qc, hi) in enumerate(seq1):
                fills, fill_at = fillmap1[(pr, qc, hi)]
                prep = (lambda j=i: qtp_prep(*seq1[j + 1])) \
                    if i + 1 < len(seq1) else (lambda: qtp_prep(2, 0, 0))
                attn_iter(pr, qc, hi,
                          pre=pre_v if (pr, qc, hi) == (0, 0, 0) else None,
                          fills=fills, fill_at=fill_at, prep=prep)

        with tc.tile_pool(name="wores", bufs=1) as wopool:
            wo_sb = wopool.tile([P, NPC, D], bf16)
            nc.sync.dma_start(wo_sb[:], wo.rearrange("(o p) n -> p o n", p=P))

            seq2 = [(pr, qc, hi) for pr in (2, 3)
                    for qc in (0, 1) for hi in (0, 1)]
            cfills = [(lambda so=so, oc=oc: c_fill(wo_sb, so, oc))
                      for so in range(NSO // 2) for oc in range(2)]
            fills3 = kq_fills(3)
            fillmap2 = {
                (2, 0, 0): (fills3[:2], 6),
                (2, 0, 1): (fills3[2:4], 6),
                (2, 1, 0): (fills3[4:6], 6),
                (2, 1, 1): (fills3[6:], 6),
                (3, 1, 0): (cfills[:4], 12),
                (3, 1, 1): (cfills[4:], 4),
            }
            for i, (pr, qc, hi) in enumerate(seq2):
                fills, fill_at = fillmap2.get((pr, qc, hi), ((), 4))
                prep = (lambda j=i: qtp_prep(*seq2[j + 1])) \
                    if i + 1 < len(seq2) else None
                attn_iter(pr, qc, hi, fills=fills, fill_at=fill_at,
                          prep=prep)
            # tail: interleave the final norm's reciprocal halves with the
            # output-projection chunks each half unblocks
            norm_step(0)
            norm_step(1)
            for so in range(NSO // 2, NSO // 2 + 4):
                for oc in range(2):
                    c_fill(wo_sb, so, oc)
            norm_step(2)
            for so in range(NSO // 2 + 4, NSO):
                for oc in range(2):
                    c_fill(wo_sb, so, oc)

    nc.compile()
    return nc


def _get_nc():
    if "nc" not in _cache:
        _cache["nc"] = _build()
    return _cache["nc"]


def make_in_maps(q, k, v, mask, Wq, bq, Wk, bk, Wv, bv, Wo, bo):
    import ml_dtypes
    f = np.float32
    bf = ml_dtypes.bfloat16
    q = np.asarray(q, dtype=f)
    k = np.asarray(k, dtype=f)
    v = np.asarray(v, dtype=f)
    Wq = np.asarray(Wq, dtype=f)
    Wk = np.asarray(Wk, dtype=f)
    Wv = np.asarray(Wv, dtype=f)
    Wo = np.asarray(Wo, dtype=f)
    bq = np.asarray(bq, dtype=f)
    bk = np.asarray(bk, dtype=f)
    mask = np.asarray(mask)

    sel = np.zeros((P, 128), dtype=f)
    sel[0, 0:64] = 1.0
    sel[32, 64:128] = 1.0

    def chunk_w(Wc):
        r = Wc.reshape(NDM, P, NPC, P)
        return np.ascontiguousarray(r.transpose(2, 1, 0, 3))

    in_maps = []
    for c in range(NCORES):
        b, hg = divmod(c, 2)
        cols = slice(hg * PROJ, (hg + 1) * PROJ)
        mbias = np.where(mask[b, 0, 0, :] == 0, f(MASK_NEG), f(0.0)).astype(f)
        in_maps.append({
            "qT": np.ascontiguousarray(q[b].T).astype(bf),
            "kT": np.ascontiguousarray(k[b].T).astype(bf),
            "vT": np.ascontiguousarray(v[b].T).astype(bf),
            "wqr": chunk_w(Wq[:, cols]).astype(bf),
            "wkr": chunk_w(Wk[:, cols]).astype(bf),
            "wv": np.ascontiguousarray(Wv[:, cols]).astype(bf),
            "wo": np.ascontiguousarray(Wo[cols, :]).astype(bf),
            "bq2": np.ascontiguousarray(bq[cols].reshape(NPC, P).T),
            "bk2": np.ascontiguousarray(bk[cols].reshape(NPC, P).T),
            "mb": np.ascontiguousarray(mbias.reshape(NKC, P).T),
            "sel": sel.astype(bf),
            "vones": np.ones((P, NSO, HPC), dtype=bf),
        })
    return in_maps


def combine_outputs(parts, bv_Wo_bo):
    bv, Wo, bo = bv_Wo_bo
    bo_eff = (np.asarray(bv, np.float32) @ np.asarray(Wo, np.float32)
              + np.asarray(bo, np.float32))
    out = np.empty((B, S, D), dtype=np.float32)
    for b in range(B):
        out[b] = (parts[2 * b].astype(np.float32)
                  + parts[2 * b + 1].astype(np.float32) + bo_eff)
    return out


def _install_axon_ntff_hook():
    import sys
    import types
    if "antenv.axon_hooks" in sys.modules:
        return
    try:
        from trn_agent_boot.trn_boot import _ntff_profile_via_ctypes
        hook = _ntff_profile_via_ctypes("/opt/axon/libaxon_pjrt.so")
    except Exception:
        hook = None
    mod = types.ModuleType("antenv.axon_hooks")
    mod._hook = hook
    mod.get_axon_ntff_profile_hook = lambda: mod._hook
    mod.set_axon_ntff_profile_hook = lambda h: setattr(mod, "_hook", h)
    sys.modules["antenv.axon_hooks"] = mod
    import concourse.bass_utils as bu
    bu.upload_artifacts = lambda tmpdir: str(tmpdir)


def kernel(q, k, v, mask, Wq, bq, Wk, bk, Wv, bv, Wo, bo):
    from concourse.bass_utils import run_bass_kernel_spmd

    nc = _get_nc()
    in_maps = make_in_maps(q, k, v, mask, Wq, bq, Wk, bk, Wv, bv, Wo, bo)
    trace = bool(int(os.environ.get("KERNEL_TRACE", "0")))
    if trace:
        try:
            _install_axon_ntff_hook()
        except Exception:
            trace = False
    try:
        res = run_bass_kernel_spmd(
            nc, in_maps, list(range(NCORES)), trace=trace,
            tmpdir=os.environ.get("KERNEL_TRACE_DIR") or None)
    except Exception:
        if not trace:
            raise
        res = run_bass_kernel_spmd(nc, in_maps, list(range(NCORES)),
                                   trace=False)
    _cache["last_result"] = res
    parts = [res.results[c]["out"] for c in range(NCORES)]
    return combine_outputs(parts, (bv, Wo, bo))
